# revision 1
# baseline (speedup 1.0000x reference)
import os

os.environ.setdefault("NEURON_CC_FLAGS", "--auto-cast=none")

import numpy as np
import jax
import jax.numpy as jnp
from functools import partial

GROUPS = 8
GP = 64
K = 64
EPS = 1e-5
N_CORES = 8

jax.config.update("jax_default_matmul_precision", "highest")


def _bn_dist(t, g, b, axes, axis_name):
    # training-mode batchnorm with cross-device batch statistics
    m = jax.lax.pmean(t.mean(axes, keepdims=True), axis_name)
    msq = jax.lax.pmean((t * t).mean(axes, keepdims=True), axis_name)
    v = msq - m * m
    shape = [1] * t.ndim
    shape[1] = -1
    return (t - m) * jax.lax.rsqrt(v + EPS) * g.reshape(shape) + b.reshape(shape)


def _fwd_impl(xn, qkv_w, bn_qkv_g, bn_qkv_b, bn_sim_g, bn_sim_b, bn_out_g, bn_out_b,
              weight, gamma, all_emb_q, all_emb_kv):
    # xn: [C, H, W] for this device's batch element n
    C, H, W = xn.shape
    B = W
    xp = xn.transpose(2, 0, 1)                            # [W, C, H] == [B, C, H]
    qkv = jnp.einsum("oc,bch->boh", qkv_w, xp)
    qkv = _bn_dist(qkv, bn_qkv_g, bn_qkv_b, (0, 2), "i")
    qkv = qkv.reshape(B, GROUPS, 2 * GP, H)
    q = qkv[:, :, : GP // 2]
    k = qkv[:, :, GP // 2 : GP]
    v = qkv[:, :, GP:]

    q_emb = all_emb_q[: GP // 2]
    k_emb = all_emb_q[GP // 2 :]
    v_emb = all_emb_kv

    qr = jnp.einsum("bgci,cij->bgij", q, q_emb)
    kr = jnp.einsum("bgci,cij->bgij", k, k_emb).transpose(0, 1, 3, 2)
    qk = jnp.einsum("bgci,bgcj->bgij", q, k)
    stacked = jnp.concatenate([qk, qr, kr], axis=1)
    stacked = _bn_dist(stacked, bn_sim_g, bn_sim_b, (0, 2, 3), "i")
    sim = jax.nn.softmax(stacked.reshape(B, 3, GROUPS, H, H).sum(axis=1), axis=3)

    sv = jnp.matmul(jnp.einsum("bgij,bgcj->bgci", sim, v), weight)
    sve = jnp.matmul(jnp.einsum("bgij,cij->bgci", sim, v_emb), weight)
    out = jnp.concatenate([sv, sve], axis=-1).reshape(B, 2 * GROUPS * GP, H)
    out = _bn_dist(out, bn_out_g, bn_out_b, (0, 2), "i")
    out = out.reshape(W, C, 2, H).sum(axis=2).transpose(1, 2, 0)  # [C, H, W]
    return xn + gamma * out


_fwd = jax.pmap(
    _fwd_impl, axis_name="i",
    in_axes=(0, None, None, None, None, None, None, None, None, None, None, None))

# variant where every arg carries a leading device axis: lets callers pre-stage
# weights on-device once (device_put_replicated) instead of re-broadcasting
_fwd_all0 = jax.pmap(_fwd_impl, axis_name="i")


def kernel(x, qkv_w, bn_qkv_g, bn_qkv_b, bn_sim_g, bn_sim_b, bn_out_g, bn_out_b,
           weight, relative, gamma, pos_map):
    x = np.asarray(x, np.float32)
    # host precompute of the static relative-position gather
    rel_idx = np.arange(K)[:, None] - np.arange(K)[None, :] + K - 1
    all_emb = np.asarray(relative)[:, rel_idx] + np.asarray(pos_map)  # [2*GP, K, K]
    all_emb_q = all_emb[:GP].astype(np.float32)      # q_emb + k_emb halves
    all_emb_kv = all_emb[GP:].astype(np.float32)     # v_emb

    out = _fwd(x,
               np.asarray(qkv_w, np.float32),
               np.asarray(bn_qkv_g, np.float32), np.asarray(bn_qkv_b, np.float32),
               np.asarray(bn_sim_g, np.float32), np.asarray(bn_sim_b, np.float32),
               np.asarray(bn_out_g, np.float32), np.asarray(bn_out_b, np.float32),
               np.asarray(weight, np.float32),
               np.float32(gamma),
               all_emb_q, all_emb_kv)
    return np.asarray(out, np.float32)



# revision 2
# speedup vs baseline: 86762157.0000x; 86762157.0000x over previous
import os

os.environ.setdefault("NEURON_CC_FLAGS", "--auto-cast=none")

import numpy as np
import jax
import jax.numpy as jnp

GROUPS = 8
GP = 64
K = 64
EPS = 1e-5
N_CORES = 8

bf16 = jnp.bfloat16
f32 = jnp.float32


def _mm(a, b):
    # tensor-engine bf16 matmul with fp32 accumulation
    return jnp.matmul(a.astype(bf16), b.astype(bf16), preferred_element_type=f32)


def _fwd_impl(xn, qkv_w, bn_qkv_g, bn_qkv_b, bn_sim_g, bn_sim_b, bn_out_g, bn_out_b,
              weight, gamma, all_emb_q, all_emb_kv):
    # xn: [C, H, W] for this device's batch element n; b == w
    C, H, W = xn.shape
    G = GROUPS

    # ---- qkv projection: one big matmul [1024,512] @ [512, H*W]
    qkv = _mm(qkv_w, xn.reshape(C, H * W))                  # [1024, H*W] f32
    # bn_qkv: per-channel stats over (b, h) across devices, folded to affine
    m = jax.lax.pmean(qkv.mean(1), "i")
    msq = jax.lax.pmean((qkv * qkv).mean(1), "i")
    s = jax.lax.rsqrt(msq - m * m + EPS) * bn_qkv_g
    qkv = qkv * s[:, None] + (bn_qkv_b - m * s)[:, None]
    qkv = qkv.reshape(G, 2 * GP, H, W)                      # o = g*128 + cc
    q = qkv[:, : GP // 2]                                   # [G, 32, i, b]
    k = qkv[:, GP // 2: GP]
    vv = qkv[:, GP:]                                        # [G, 64, j, b]

    q_emb = all_emb_q[: GP // 2]                            # [32, i, j]
    k_emb = all_emb_q[GP // 2:]
    v_emb = all_emb_kv                                      # [64, i, j]

    # ---- attention logits: all as batched matmuls
    # qk[g,b,i,j] = sum_c q[g,c,i,b] k[g,c,j,b]
    qk = _mm(q.transpose(0, 3, 2, 1), k.transpose(0, 3, 1, 2))      # [g,b,i,j]
    # qr[g,b,i,j] = sum_c q[g,c,i,b] q_emb[c,i,j]  (batch over i)
    qT = q.transpose(2, 0, 3, 1).reshape(H, G * W, GP // 2)         # [i,(g,b),c]
    qr = _mm(qT, q_emb.transpose(1, 0, 2))                          # [i,(g,b),j]
    qr = qr.reshape(H, G, W, H).transpose(1, 2, 0, 3)               # [g,b,i,j]
    # kr[g,b,i,j] = sum_c k[g,c,j,b] k_emb[c,j,i]  (batch over j)
    kT = k.transpose(2, 0, 3, 1).reshape(H, G * W, GP // 2)         # [j,(g,b),c]
    kr = _mm(kT, k_emb.transpose(1, 0, 2))                          # [j,(g,b),i]
    kr = kr.reshape(H, G, W, H).transpose(1, 2, 3, 0)               # [g,b,i,j]

    # ---- bn_sim folded: softmax is invariant to the per-channel additive
    # constants, so only the per-(tensor, g) scales matter
    def ch_stats(t):
        mm_ = jax.lax.pmean(t.mean((1, 2, 3)), "i")
        ms_ = jax.lax.pmean((t * t).mean((1, 2, 3)), "i")
        return jax.lax.rsqrt(ms_ - mm_ * mm_ + EPS)
    aqk = ch_stats(qk) * bn_sim_g[0:8]
    aqr = ch_stats(qr) * bn_sim_g[8:16]
    akr = ch_stats(kr) * bn_sim_g[16:24]
    logits = (qk * aqk[:, None, None, None] + qr * aqr[:, None, None, None]
              + kr * akr[:, None, None, None])
    sim = jax.nn.softmax(logits, axis=3)                            # [g,b,i,j]

    # ---- sv / sve
    svp = _mm(sim, vv.transpose(0, 3, 2, 1))                        # [g,b,i,c]
    sv = _mm(svp.transpose(0, 1, 3, 2), weight)                     # [g,b,c,m]
    simT = sim.transpose(2, 0, 1, 3).reshape(H, G * W, H)           # [i,(g,b),j]
    svep = _mm(simT, v_emb.transpose(1, 2, 0))                      # [i,(g,b),c]
    svep = svep.reshape(H, G, W, GP).transpose(1, 2, 3, 0)          # [g,b,c,i]
    sve = _mm(svep, weight)                                         # [g,b,c,m]

    # ---- bn_out (channel = (g*64+c)*2 + {0:sv,1:sve}), then pair-sum + residual
    def out_stats(t):
        mm_ = jax.lax.pmean(t.mean((1, 3)), "i")                    # [g,c]
        ms_ = jax.lax.pmean((t * t).mean((1, 3)), "i")
        return mm_, jax.lax.rsqrt(ms_ - mm_ * mm_ + EPS)
    msv, rsv = out_stats(sv)
    msve, rsve = out_stats(sve)
    gch = bn_out_g.reshape(G, GP, 2)
    bch = bn_out_b.reshape(G, GP, 2)
    ssv = rsv * gch[:, :, 0]
    ssve = rsve * gch[:, :, 1]
    const = (bch[:, :, 0] - msv * ssv) + (bch[:, :, 1] - msve * ssve)
    branch = (sv * ssv[:, None, :, None] + sve * ssve[:, None, :, None]
              + const[:, None, :, None])                            # [g,b,c,m]
    branch = branch.transpose(0, 2, 3, 1).reshape(C, H, W)          # [(g,c), m, b]
    return xn + gamma * branch


_fwd = jax.pmap(
    _fwd_impl, axis_name="i",
    in_axes=(0, None, None, None, None, None, None, None, None, None, None, None))

_fwd_all0 = jax.pmap(_fwd_impl, axis_name="i")


def kernel(x, qkv_w, bn_qkv_g, bn_qkv_b, bn_sim_g, bn_sim_b, bn_out_g, bn_out_b,
           weight, relative, gamma, pos_map):
    x = np.asarray(x, np.float32)
    rel_idx = np.arange(K)[:, None] - np.arange(K)[None, :] + K - 1
    all_emb = np.asarray(relative)[:, rel_idx] + np.asarray(pos_map)
    all_emb_q = all_emb[:GP].astype(np.float32)
    all_emb_kv = all_emb[GP:].astype(np.float32)

    out = _fwd(x,
               np.asarray(qkv_w, np.float32),
               np.asarray(bn_qkv_g, np.float32), np.asarray(bn_qkv_b, np.float32),
               np.asarray(bn_sim_g, np.float32), np.asarray(bn_sim_b, np.float32),
               np.asarray(bn_out_g, np.float32), np.asarray(bn_out_b, np.float32),
               np.asarray(weight, np.float32),
               np.float32(gamma),
               all_emb_q, all_emb_kv)
    return np.asarray(out, np.float32)
